# revision 32
# baseline (speedup 1.0000x reference)
"""GQA kernel for trn2, 8 NeuronCores — bf16 version.

Problem: B=2, N=2048, d_model=2048, 32 q heads / 8 kv heads, d_head=64.
Sharding: batch (2) x head-groups (4): core c = b*4 + g handles batch b and
q heads [8g, 8g+8) (kv heads [2g, 2g+1]).  Each core computes
partial_out = attn_out_g @ Wo[:, cols_g].T ; host sums the 4 group partials
per batch and adds bo.

Key differences vs the fp32 baseline:
  * all matmul operands bf16 (1 cyc/row on PE vs 4 for fp32)
  * x is transposed on the host (xT input) — no PE transposes
  * softmax denominator fused into the AV matmul via a ones column in the
    65-wide V' stationary (no separate ones-matmul per key chunk)
  * exp over [128,1024] PSUM (both head halves at once) to halve ACT
    instruction count
  * out-projection of q-tile qt-1 interleaved into attention of qt to
    fill PE bubbles in the ACT-bound inner loop

Per-core layouts (bf16 unless noted):
  xT_sb [128, 16*2048]  chunk ck = xT rows [128ck,128ck+128), free = tokens
  qT  [128, 4*2048]  chunk j holds heads (j, j+4): partitions 0:64 = head j
                     dims, 64:128 = head j+4 dims; free = tokens.
  kT  [128, 2048]    partitions 0:64 = kv0 k-dims, 64:128 = kv1 k-dims.
  vp0/vp1 [128, 16*65]  chunk kc = [64 v-dims | ones]: AV out rows 0:64 =
                     attn numerator, row 64 = denominator.  Half-1 data is
                     partition-shifted 0:64 -> 64:128 by a PSUM->SBUF DMA
                     before the normalize multiply (PE out base partitions
                     must be 0/32/64, so it can't land at 63:128 directly).
"""

import numpy as np
import ml_dtypes

import concourse.bass as bass
import concourse.mybir as mybir
from concourse.tile import TileContext, add_dep_helper
from concourse.bass_utils import run_bass_kernel_spmd



def _split_matmul_waits(bir_bytes):
    """Walrus in this toolchain allows only ONE sync wait per Matmult.

    For any matmul carrying N>1 waits, insert a PE NoOp immediately
    before it holding the first N-1 waits; the matmul keeps the last.
    The NoOp precedes the matmul in the PE stream, so ordering
    semantics are identical.
    """
    import json as _json
    bir = _json.loads(bir_bytes)
    n = 0
    for f in bir["functions"]:
        for b in f["blocks"]:
            out = []
            for i in b["instructions"]:
                si = i.get("sync_info") if isinstance(i, dict) else None
                eng = i.get("engine") if isinstance(i, dict) else None
                if (si and len(si.get("on_wait", [])) > 1
                        and eng and eng != "Unassigned"):
                    waits = si["on_wait"]
                    for w in waits[:-1]:
                        out.append({
                            "debug": i.get("debug", 0),
                            "engine": eng,
                            "ins": [], "outs": [],
                            "name": "%s-w%d" % (i["name"], n),
                            "opcode": "NoOp",
                            "sync_info": {"on_update": [], "on_wait": [w]},
                        })
                        n += 1
                    si["on_wait"] = waits[-1:]
                out.append(i)
            b["instructions"] = out
    return _json.dumps(bir).encode()


def _pe_touch(nc, producers):
    """Advance PE's vector clock past each producer, one sem at a time."""
    for p in producers:
        n = nc.tensor.nop()
        add_dep_helper(n.ins, p.ins, sync=True, reason="pe-wait-absorber")


F32 = mybir.dt.float32
BF16 = mybir.dt.bfloat16
AF = mybir.ActivationFunctionType

D = 2048      # d_model
TOKS = 2048   # tokens per batch
QD = 512      # q dims per core
DH = 64
NCK = 16      # d_model chunks of 128
TT = 512      # token tile for projections
NTT = TOKS // TT
QTILE = 512
NQT = TOKS // QTILE
NKC = TOKS // 128   # key chunks of 128
V65 = DH + 1        # V' stationary width incl. ones column
SCALE = DH ** -0.5  # 0.125

# local head order within a core: chunk j holds heads (j, j+4)
HEAD_ORDER = [0, 4, 1, 5, 2, 6, 3, 7]


def _build():
    nc = bass.Bass()
    # all big inputs arrive pre-arranged by the host as their exact SBUF
    # images ([128 partitions, free]) so each loads in ONE DMA dispatch —
    # per-dispatch descriptor generation (~600ns each) was gating the
    # projection phase, not bandwidth
    xT = nc.declare_dram_parameter("xT", [128, NCK * TOKS], BF16,
                                   isOutput=False)
    wqT = nc.declare_dram_parameter("wqT", [128, NCK * QD], BF16,
                                    isOutput=False)
    wkT = nc.declare_dram_parameter("wkT", [128, NCK * 128], BF16,
                                    isOutput=False)
    wvT = nc.declare_dram_parameter("wvT", [128, NCK * 128], BF16,
                                    isOutput=False)
    woT = nc.declare_dram_parameter("woT", [128, 4 * D], BF16,
                                    isOutput=False)
    bq4 = nc.declare_dram_parameter("bq4", [128, 4], F32, isOutput=False)
    bkT = nc.declare_dram_parameter("bkT", [128, 1], F32, isOutput=False)
    bvT = nc.declare_dram_parameter("bvT", [128, 1], F32, isOutput=False)
    eye = nc.declare_dram_parameter("eye", [128, 128], BF16, isOutput=False)
    out = nc.declare_dram_parameter("out", [TOKS, D], BF16, isOutput=True)
    scr = nc.declare_dram_parameter("scr", [4, 2 * QTILE], BF16,
                                    isOutput=True)

    with TileContext(nc) as tc, \
            nc.allow_low_precision(reason="bf16 kernel; tol 2e-2"), \
            tc.tile_pool(name="persist", bufs=1) as pp:
        if True:
            xT_sb = pp.tile([128, NCK * TOKS], BF16, tag="xT")
            wq_sb = pp.tile([128, NCK * QD], BF16, tag="wq")
            wk_sb = pp.tile([128, NCK * 128], BF16, tag="wk")
            wv_sb = pp.tile([128, NCK * 128], BF16, tag="wv")
            wo_sb = pp.tile([128, 4 * D], BF16, tag="wo")
            qT = pp.tile([128, 4 * TOKS], BF16, tag="qT")
            kT = pp.tile([128, TOKS], BF16, tag="kT")
            vT_sb = pp.tile([128, TOKS], BF16, tag="vT")
            vp0 = pp.tile([128, NKC * V65], BF16, tag="vp0")
            vp1 = pp.tile([128, NKC * V65], BF16, tag="vp1")
            ones65 = pp.tile([65, DH], BF16, tag="ones65")
            eye_sb = pp.tile([128, 128], BF16, tag="eye")
            bq_sb = pp.tile([128, 4], F32, tag="bq")
            bk_sb = pp.tile([128, 1], F32, tag="bk")
            bv_sb = pp.tile([128, 1], F32, tag="bv")

            # input loads: wk/wv first (small, gate the first K/V matmuls),
            # then xT in 16 single-chunk slices round-robin over FOUR DMA
            # queues so chunks land well ahead of the projection loop's
            # consumption order; wq/wo (big, needed later) go last.
            # queue plan (3 DMA queues, each ~130GB/s, depth ~4): xT chunks
            # round-robin in consumption order; wk/wv ahead of their first
            # use; wq is jg-major so Q(tt0,j0) only needs the first quarter
            # (lands right as K/V projections finish); biases/eye/wo slot
            # into remaining queue time before their ~30-100us uses.
            # xT arrives tt-major ([128, tt(4) x ck(16) x 512]) in 16
            # quarter-slab dispatches so the K/V projections (also
            # tt-major now) stream behind the DMA with no bulk wait
            # gpsimd/scalar carry only xT slabs (slab0/slab1 land first);
            # sync carries wk+wv up front, then every third slab, then the
            # later-needed weights.  wq's j0 block is slotted so it lands
            # just before Q(tt0,j0) follows the K/V projections.
            nc.sync.dma_start(out=wk_sb[:, :], in_=wkT[:, :])
            SLAB = 4 * TT  # 4 ck-slices of one tt
            xqueues = [nc.gpsimd, nc.scalar, nc.sync]
            for s in range(16):
                if s == 0:
                    nc.gpsimd.dma_start(out=xT_sb[:, 0:SLAB],
                                        in_=xT[:, 0:SLAB])
                    nc.gpsimd.dma_start(out=wv_sb[:, :], in_=wvT[:, :])
                    continue
                xqueues[s % 3].dma_start(
                    out=xT_sb[:, s * SLAB:(s + 1) * SLAB],
                    in_=xT[:, s * SLAB:(s + 1) * SLAB])
                if s == 5:
                    nc.scalar.dma_start(out=wq_sb[:, 0:NCK * 128],
                                        in_=wqT[:, 0:NCK * 128])
                    nc.sync.dma_start(out=bk_sb[:, :], in_=bkT[:, :])
                    nc.gpsimd.dma_start(out=bv_sb[:, :], in_=bvT[:, :])
                    nc.sync.dma_start(out=eye_sb[:, :], in_=eye[:, :])
            nc.scalar.dma_start(out=bq_sb[:, :], in_=bq4[:, :])
            nc.gpsimd.dma_start(out=wq_sb[:, NCK * 128:],
                                in_=wqT[:, NCK * 128:])
            nc.sync.dma_start(out=wo_sb[:, :], in_=woT[:, :])
            # preload the Exp activation table so the first real exp
            # doesn't eat the ~1.3us table-load latency mid-pipeline
            warm = pp.tile([1, 1], F32, tag="warm")
            nc.vector.memset(warm[:, :], 0.0)
            nc.scalar.activation(warm[:, :], warm[:, :], AF.Exp)
            # ones columns of vp0/vp1 and ones65: memset whole tiles to 1.0;
            # V-proj writes only the 64 data columns of each 65-chunk.
            nc.vector.memset(vp0[:, :], 1.0)
            nc.vector.memset(vp1[:, :], 1.0)
            nc.vector.memset(ones65[:, :], 1.0)

            # ---------------- projection phase ----------------
            # K and vT interleaved per contraction chunk so the PE has
            # work as soon as each xT chunk's DMA lands (8 PSUM banks:
            # one accumulator per token tile per projection).  Q is done
            # here only for (tt=0, j=0); the rest streams into the
            # attention rounds via the quota queue.
            with tc.tile_pool(name="proj", bufs=1) as jp, \
                 tc.tile_pool(name="projps", bufs=1, space="PSUM") as jpp:
                # tt-major: each 512-token slab's K/V accumulation streams
                # right behind its DMA slices, then its V' transposes run,
                # so the PE starts ~10us in and never bulk-waits on xT
                kps = [jpp.tile([128, TT], F32, tag="kp", bufs=4,
                                name="kp%d" % t) for t in range(NTT)]
                vts = [jpp.tile([128, TT], F32, tag="vt", bufs=4,
                                name="vt%d" % t) for t in range(NTT)]
                TTW = NCK * TT  # cols per tt block in the tt-major image
                for tt in range(NTT):
                    for ck in range(NCK):
                        nc.tensor.matmul(
                            kps[tt][:, :],
                            wk_sb[:, ck * 128:(ck + 1) * 128],
                            xT_sb[:, tt * TTW + ck * TT:
                                  tt * TTW + (ck + 1) * TT],
                            start=(ck == 0), stop=(ck == NCK - 1))
                        nc.tensor.matmul(
                            vts[tt][:, :],
                            wv_sb[:, ck * 128:(ck + 1) * 128],
                            xT_sb[:, tt * TTW + ck * TT:
                                  tt * TTW + (ck + 1) * TT],
                            start=(ck == 0), stop=(ck == NCK - 1))
                    nc.vector.tensor_scalar_add(
                        kT[:, tt * TT:(tt + 1) * TT], kps[tt][:, :],
                        bk_sb[:, 0:1])
                    nc.vector.tensor_scalar_add(
                        vT_sb[:, tt * TT:(tt + 1) * TT], vts[tt][:, :],
                        bv_sb[:, 0:1])
                    # V' natural layout [keys, v-dims] via PE transpose
                    for kc in range(4 * tt, 4 * tt + 4):
                        tp = jpp.tile([128, 128], BF16, tag="vt", bufs=4)
                        nc.tensor.transpose(
                            tp[:, :], vT_sb[:, kc * 128:(kc + 1) * 128],
                            eye_sb[:, :])
                        nc.vector.tensor_copy(
                            vp0[:, kc * V65: kc * V65 + DH], tp[:, 0:DH])
                        nc.vector.tensor_copy(
                            vp1[:, kc * V65: kc * V65 + DH],
                            tp[:, DH:128])
                # Q(tt0,j0) reuses the kp banks (tag sharing — no
                # pool-close barrier before attention)
                ps = jpp.tile([128, TT], F32, tag="kp", bufs=4)
                for ck in range(NCK):
                    nc.tensor.matmul(
                        ps[:, :],
                        wq_sb[:, ck * 128: (ck + 1) * 128],
                        xT_sb[:, ck * TT: (ck + 1) * TT],
                        start=(ck == 0), stop=(ck == NCK - 1))
                nc.vector.tensor_scalar_add(
                    qT[:, 0:TT], ps[:, :], bq_sb[:, 0:1])

            # ---------------- attention + out-projection ----------------
            with tc.tile_pool(name="attn", bufs=1) as ap, \
                 tc.tile_pool(name="attnps", bufs=1, space="PSUM") as app:

                def op_store(qt, n, m, op, engs=None, osbufs=4):
                    osb = ap.tile([128, 512], BF16, tag="osb", bufs=osbufs)
                    nc.vector.tensor_copy(osb[:, :], op[:, :])
                    # alternate store queues so writeback never queues
                    # behind the normalize chain's small sync-queue DMAs
                    engs = engs or (nc.sync, nc.gpsimd)
                    eng = engs[(n * 4 + m) % len(engs)]
                    eng.dma_start(
                        out=out[qt * QTILE + m * 128:
                                qt * QTILE + (m + 1) * 128,
                                n * 512:(n + 1) * 512],
                        in_=osb[:, :])

                def op_group(qt, n, m, oT_t, pool, tag, bufs):
                    # out[qt*512+m*128 : +128, n*512 : +512] partial
                    op = pool.tile([128, TT], F32, tag=tag, bufs=bufs)
                    for j in range(4):
                        nc.tensor.matmul(
                            op[:, :],
                            oT_t[:, j * QTILE + m * 128:
                                 j * QTILE + (m + 1) * 128],
                            wo_sb[:, j * D + n * 512: j * D + (n + 1) * 512],
                            start=(j == 0), stop=(j == 3))
                    op_store(qt, n, m, op)

                def spair(qt, j, kc):
                    Sp = app.tile([128, 2 * QTILE], F32, tag="S", bufs=2)
                    qs = j * TOKS + qt * QTILE
                    nc.tensor.matmul(
                        Sp[:, 0:QTILE],
                        kT[0:64, kc * 128:(kc + 1) * 128],
                        qT[0:64, qs:qs + QTILE],
                        start=True, stop=True)
                    nc.tensor.matmul(
                        Sp[:, QTILE:2 * QTILE],
                        kT[64:128, kc * 128:(kc + 1) * 128],
                        qT[64:128, qs:qs + QTILE],
                        start=True, stop=True)
                    return Sp

                # Normalize oT = num/den.  Split into deferred stages so
                # the slow parts never block the PE queue or the opj banks:
                #   kc==15: stage PSUM out to SBUF (bf16, incl. den row),
                #           spread den [1,1024] -> [8,128] via DMA so the
                #           multi-pass DVE reciprocal costs ~0.9us not 6.5
                #   +1:     reciprocal on the spread layout
                #   +2:     cast + gather back to [1,1024] + half-1
                #           partition-shift DMA
                #   +7:     PE den broadcast, bcs copy, muls into oT
                def norm_stage1(opj0, opj1, pool=None):
                    pool = pool or ap
                    stg65 = pool.tile([65, 2 * QTILE], BF16, tag="stg65",
                                      bufs=2)
                    nc.vector.tensor_copy(stg65[:, 0:QTILE], opj0[0:65, :])
                    nc.vector.tensor_copy(stg65[:, QTILE:2 * QTILE],
                                          opj1[0:65, :])
                    dsp = pool.tile([8, 128], BF16, tag="dsp", bufs=2)
                    nc.sync.dma_start(out=dsp[0:8, 0:128],
                                      in_=stg65[64:65, 0:2 * QTILE])
                    return stg65, dsp

                def norm_recip(ctx, pool=None):
                    pool = pool or ap
                    stg65, dsp = ctx["stg"]
                    rdf = pool.tile([8, 128], F32, tag="rdf", bufs=2)
                    nc.vector.reciprocal(rdf[:, :], dsp[:, :])
                    ctx["rdf"] = rdf

                def norm_gather(ctx, pool=None):
                    pool = pool or ap
                    stg65, dsp = ctx["stg"]
                    rdb = pool.tile([8, 128], BF16, tag="rdb", bufs=2)
                    nc.vector.tensor_copy(rdb[:, :], ctx["rdf"][:, :])
                    # reciprocal row -> DRAM scratch -> broadcast-read to all
                    # 128 partitions: replaces the two PE broadcast matmuls
                    s = norm_gather.seg % 4
                    norm_gather.seg += 1
                    nc.sync.dma_start(out=scr[s:s + 1, :], in_=rdb[0:8, 0:128])
                    bcs = pool.tile([128, 2 * QTILE], BF16, tag="bcsb",
                                    bufs=2)
                    nc.sync.dma_start(
                        out=bcs[:, :],
                        in_=scr[s:s + 1, :].partition_broadcast(128))
                    stgB = pool.tile([128, QTILE], BF16, tag="stgB", bufs=2)
                    nc.sync.dma_start(out=stgB[64:128, :],
                                      in_=stg65[0:64, QTILE:2 * QTILE])
                    ctx["bcs"], ctx["stgB"] = bcs, stgB
                norm_gather.seg = 0

                def norm_stage2(ctx, oT_t, j, pspool=None, pstag="op",
                                psbufs=1, sbpool=None):
                    stg65, _ = ctx["stg"]
                    bcs, stgB = ctx["bcs"], ctx["stgB"]
                    nc.vector.tensor_mul(
                        oT_t[0:64, j * QTILE:(j + 1) * QTILE],
                        stg65[0:64, 0:QTILE], bcs[0:64, 0:QTILE])
                    nc.vector.tensor_mul(
                        oT_t[64:128, j * QTILE:(j + 1) * QTILE],
                        stgB[64:128, :], bcs[64:128, QTILE:2 * QTILE])

                # software-pipelined emission: per round, ACT gets exp(k)
                # first, then PE gets S-pair(k+1), an out-proj filler
                # group, the AV-pair(k), and any deferred normalize stages
                # scheduled for this round.
                rounds = [(qt, j, kc) for qt in range(NQT)
                          for j in range(4) for kc in range(NKC)]
                sched = {}  # round index -> [closure]
                pending, prev_ops, pi = [], [], 0
                oT_sb = None
                opj0 = opj1 = None
                # deferred Q-projection jobs (token tiles 1..3), paced into
                # attention rounds: 2/round in qt0 (no out-proj fillers
                # there), 1/round in qt1-2 skipping filler rounds, so
                # per-round PE work stays under the ACT exp cadence
                qjobs = [(0, jg, ck) for jg in range(1, 4)
                         for ck in range(NCK)]
                qjobs += [(tt, jg, ck) for tt in range(1, NTT)
                          for jg in range(4) for ck in range(NCK)]
                qjobs.reverse()  # pop() from the front order
                qp2 = None

                def emit_qmm(job):
                    nonlocal qp2
                    tt, jg, ck = job
                    if ck == 0:
                        qp2 = app.tile([128, TT], F32, tag="qp2", bufs=1)
                    nc.tensor.matmul(
                        qp2[:, :],
                        wq_sb[:, jg * (NCK * 128) + ck * 128:
                              jg * (NCK * 128) + (ck + 1) * 128],
                        xT_sb[:, tt * TTW + ck * TT:
                              tt * TTW + (ck + 1) * TT],
                        start=(ck == 0), stop=(ck == NCK - 1))
                    if ck == NCK - 1:
                        nc.vector.tensor_scalar_add(
                            qT[:, jg * TOKS + tt * TT:
                               jg * TOKS + (tt + 1) * TT],
                            qp2[:, :], bq_sb[:, jg:jg + 1])

                Sp_cur = spair(*rounds[0])
                for i, (qt, j, kc) in enumerate(rounds):
                    if j == 0 and kc == 0:
                        # persist pool: read by the tail-pool out-proj after
                        # the attn SBUF pool closes
                        oT_sb = pp.tile([128, 4 * QTILE], BF16, tag="oTsb",
                                        bufs=2)
                        prev_ops, pi = pending, 0
                        pending = [(qt, n, m, oT_sb)
                                   for n in range(4) for m in range(4)]
                    if kc == 0:
                        opj0 = app.tile([65, QTILE], F32, tag="opj0", bufs=1)
                        opj1 = app.tile([65, QTILE], F32, tag="opj1", bufs=1)
                    E = ap.tile([128, 2 * QTILE], BF16, tag="E", bufs=4)
                    nc.scalar.activation(
                        E[:, :], Sp_cur[:, :], AF.Exp, scale=SCALE)
                    Sp_nxt = (spair(*rounds[i + 1])
                              if i + 1 < len(rounds) else None)
                    r = j * NKC + kc
                    did_op = False
                    if r >= 8 and r % 3 == 2 and pi < len(prev_ops):
                        op_group(*prev_ops[pi], app, "op", 1)
                        pi += 1
                        did_op = True
                    for fn in sched.pop(i, ()):
                        fn()
                    # 2/round in qt0 (no fillers there), 1/round in
                    # non-filler rounds after — keeps per-round PE work
                    # near the ACT exp cadence while finishing each token
                    # tile's Q before its attention q-tile starts
                    quota = 2 if qt == 0 else (0 if did_op else 1)
                    while quota > 0 and qjobs:
                        emit_qmm(qjobs.pop())
                        quota -= 1
                    # AV: stationary [v-dims | ones] -> rows 0:65
                    # (row 64 = denominator)
                    nc.tensor.matmul(
                        opj0[0:V65, :],
                        vp0[:, kc * V65:(kc + 1) * V65],
                        E[:, 0:QTILE],
                        start=(kc == 0), stop=(kc == NKC - 1))
                    nc.tensor.matmul(
                        opj1[0:V65, :],
                        vp1[:, kc * V65:(kc + 1) * V65],
                        E[:, QTILE:2 * QTILE],
                        start=(kc == 0), stop=(kc == NKC - 1))
                    if kc == NKC - 1:
                        if i == len(rounds) - 1:
                            # last segment: stage1 tiles live in the persist
                            # pool so the remaining stages can run in the
                            # tail pool after the attn pools close
                            tail_ctx = {"stg": norm_stage1(opj0, opj1,
                                                           pool=pp)}
                            tail_j = j
                        else:
                            ctx = {"stg": norm_stage1(opj0, opj1)}
                            sched.setdefault(i + 1, []).append(
                                lambda c=ctx: norm_recip(c))
                            sched.setdefault(i + 2, []).append(
                                lambda c=ctx: norm_gather(c))
                            sched.setdefault(i + 7, []).append(
                                lambda c=ctx, t=oT_sb, jj=j:
                                norm_stage2(c, t, jj))
                    Sp_cur = Sp_nxt
                # flush deferred normalize stages of the last segment
                for idx in sorted(sched):
                    for fn in sched[idx]:
                        fn()
                while pi < len(prev_ops):
                    op_group(*prev_ops[pi], app, "op", 1)
                    pi += 1
            # tail: qt3's out-projection. The PE queue is in-order, so the
            # j3-normalize chain's DVE/DMA hops go first, then 7 groups'
            # j0..j2 partial matmuls keep the PE streaming while that chain
            # completes; stage2(j3) then lands with no PE stall, followed by
            # the j3 finishers and the remaining 9 full groups.
            with tc.tile_pool(name="tail", bufs=1) as ap, \
                 tc.tile_pool(name="tailps", bufs=1, space="PSUM") as tpp:
                norm_recip(tail_ctx, pool=ap)
                norm_gather(tail_ctx, pool=ap)

                def op_partial(args):
                    (qt, n, m, oT_t) = args
                    op = tpp.tile([128, TT], F32, tag="opt", bufs=8)
                    for j in range(3):
                        nc.tensor.matmul(
                            op[:, :],
                            oT_t[:, j * QTILE + m * 128:
                                 j * QTILE + (m + 1) * 128],
                            wo_sb[:, j * D + n * 512: j * D + (n + 1) * 512],
                            start=(j == 0), stop=False)
                    return op

                tailq = (nc.gpsimd, nc.scalar, nc.sync)
                ops1 = [op_partial(a) for a in pending[:8]]
                # bcj(j3) sits here in the in-order PE stream: by now the
                # 18 partial matmuls above have covered the recip chain
                norm_stage2(tail_ctx, oT_sb, tail_j, pspool=tpp,
                            pstag="bcj", psbufs=1, sbpool=ap)
                for (qt, n, m, oT_t), op in zip(pending[:8], ops1):
                    nc.tensor.matmul(
                        op[:, :],
                        oT_t[:, 3 * QTILE + m * 128:
                             3 * QTILE + (m + 1) * 128],
                        wo_sb[:, 3 * D + n * 512: 3 * D + (n + 1) * 512],
                        start=False, stop=True)
                    op_store(qt, n, m, op, engs=tailq, osbufs=6)
                for (qt, n, m, oT_t) in pending[8:]:
                    op = tpp.tile([128, TT], F32, tag="opt", bufs=8)
                    for j in range(4):
                        nc.tensor.matmul(
                            op[:, :],
                            oT_t[:, j * QTILE + m * 128:
                                 j * QTILE + (m + 1) * 128],
                            wo_sb[:, j * D + n * 512: j * D + (n + 1) * 512],
                            start=(j == 0), stop=(j == 3))
                    op_store(qt, n, m, op, engs=tailq, osbufs=6)
    return nc


def _prep_inputs(x, Wq, bq, Wk, bk, Wv, bv, Wo, bo):
    """Build the 8 per-core input maps."""
    f = np.float32
    bf = ml_dtypes.bfloat16
    x = np.asarray(x, f)
    Wq, bq = np.asarray(Wq, f), np.asarray(bq, f)
    Wk, bk = np.asarray(Wk, f), np.asarray(bk, f)
    Wv, bv = np.asarray(Wv, f), np.asarray(bv, f)
    Wo = np.asarray(Wo, f)
    # per-core head-dim permutation within the group's 512 q dims
    perm = np.concatenate([
        np.arange(h * DH, (h + 1) * DH) for h in HEAD_ORDER])
    eye = np.eye(128, dtype=f).astype(bf)
    in_maps = []
    for c in range(8):
        b, g = divmod(c, 4)
        wq_g = Wq[g * QD:(g + 1) * QD, :][perm, :]     # (512, 2048)
        bq_g = bq[g * QD:(g + 1) * QD][perm]
        wo_g = Wo[:, g * QD:(g + 1) * QD].T[perm, :]   # (512, 2048)
        def sbimg(a):
            # [NCK*128, w] -> SBUF image [128, NCK*w]: chunk ck of 128
            # DRAM rows becomes columns [ck*w, (ck+1)*w) on partition p
            a = np.asarray(a, f).astype(bf)
            n, w = a.shape
            return np.ascontiguousarray(
                a.reshape(n // 128, 128, w).transpose(1, 0, 2)
                .reshape(128, (n // 128) * w))

        # wq image is jg-major: [128, jg(4) x ck(16) x 128] so the j0 block
        # (first quarter) alone unblocks Q(tt0, j0)
        wq_jg = (np.asarray(wq_g.T, f).astype(bf)
                 .reshape(16, 128, 4, 128).transpose(1, 2, 0, 3)
                 .reshape(128, 4 * 16 * 128))
        # xT image is tt-major: [128, tt(4) x ck(16) x 512]
        xT_tt = (x[b].T.astype(bf).reshape(16, 128, 4, 512)
                 .transpose(1, 2, 0, 3).reshape(128, 4 * 16 * 512))
        in_maps.append({
            "xT": np.ascontiguousarray(xT_tt),
            "wqT": np.ascontiguousarray(wq_jg),
            "wkT": sbimg(Wk[g * 128:(g + 1) * 128, :].T),
            "wvT": sbimg(Wv[g * 128:(g + 1) * 128, :].T),
            "woT": sbimg(wo_g),
            "bq4": np.ascontiguousarray(bq_g.reshape(4, 128).T),
            "bkT": np.ascontiguousarray(bk[g * 128:(g + 1) * 128, None]),
            "bvT": np.ascontiguousarray(bv[g * 128:(g + 1) * 128, None]),
            "eye": eye,
        })
    return in_maps


def run(inputs, trace=False, **kw):
    nc = _build()
    _orig_tjb = nc.to_json_bytes
    nc.to_json_bytes = lambda: _split_matmul_waits(_orig_tjb())
    in_maps = _prep_inputs(**inputs)
    res = run_bass_kernel_spmd(nc, in_maps, list(range(8)), trace=trace, **kw)
    bo = np.asarray(inputs["bo"], np.float32)
    y = np.empty((2, TOKS, D), np.float32)
    for b in range(2):
        acc = res.results[4 * b]["out"].astype(np.float32)
        for g in range(1, 4):
            acc = acc + res.results[4 * b + g]["out"].astype(np.float32)
        y[b] = acc + bo[None, :]
    return y, res


def kernel(**inputs):
    y, _ = run(inputs, trace=False)
    return y



# revision 33
# speedup vs baseline: 1.0052x; 1.0052x over previous
"""GQA kernel for trn2, 8 NeuronCores — bf16 version.

Problem: B=2, N=2048, d_model=2048, 32 q heads / 8 kv heads, d_head=64.
Sharding: batch (2) x head-groups (4): core c = b*4 + g handles batch b and
q heads [8g, 8g+8) (kv heads [2g, 2g+1]).  Each core computes
partial_out = attn_out_g @ Wo[:, cols_g].T ; host sums the 4 group partials
per batch and adds bo.

Key differences vs the fp32 baseline:
  * all matmul operands bf16 (1 cyc/row on PE vs 4 for fp32)
  * x is transposed on the host (xT input) — no PE transposes
  * softmax denominator fused into the AV matmul via a ones column in the
    65-wide V' stationary (no separate ones-matmul per key chunk)
  * exp over [128,1024] PSUM (both head halves at once) to halve ACT
    instruction count
  * out-projection of q-tile qt-1 interleaved into attention of qt to
    fill PE bubbles in the ACT-bound inner loop

Per-core layouts (bf16 unless noted):
  xT_sb [128, 16*2048]  chunk ck = xT rows [128ck,128ck+128), free = tokens
  qT  [128, 4*2048]  chunk j holds heads (j, j+4): partitions 0:64 = head j
                     dims, 64:128 = head j+4 dims; free = tokens.
  kT  [128, 2048]    partitions 0:64 = kv0 k-dims, 64:128 = kv1 k-dims.
  vp0/vp1 [128, 16*65]  chunk kc = [64 v-dims | ones]: AV out rows 0:64 =
                     attn numerator, row 64 = denominator.  Half-1 data is
                     partition-shifted 0:64 -> 64:128 by a PSUM->SBUF DMA
                     before the normalize multiply (PE out base partitions
                     must be 0/32/64, so it can't land at 63:128 directly).
"""

import numpy as np
import ml_dtypes

import concourse.bass as bass
import concourse.mybir as mybir
from concourse.tile import TileContext, add_dep_helper
from concourse.bass_utils import run_bass_kernel_spmd



def _split_matmul_waits(bir_bytes):
    """Walrus in this toolchain allows only ONE sync wait per Matmult.

    For any matmul carrying N>1 waits, insert a PE NoOp immediately
    before it holding the first N-1 waits; the matmul keeps the last.
    The NoOp precedes the matmul in the PE stream, so ordering
    semantics are identical.
    """
    import json as _json
    bir = _json.loads(bir_bytes)
    n = 0
    for f in bir["functions"]:
        for b in f["blocks"]:
            out = []
            for i in b["instructions"]:
                si = i.get("sync_info") if isinstance(i, dict) else None
                eng = i.get("engine") if isinstance(i, dict) else None
                if (si and len(si.get("on_wait", [])) > 1
                        and eng and eng != "Unassigned"):
                    waits = si["on_wait"]
                    for w in waits[:-1]:
                        out.append({
                            "debug": i.get("debug", 0),
                            "engine": eng,
                            "ins": [], "outs": [],
                            "name": "%s-w%d" % (i["name"], n),
                            "opcode": "NoOp",
                            "sync_info": {"on_update": [], "on_wait": [w]},
                        })
                        n += 1
                    si["on_wait"] = waits[-1:]
                out.append(i)
            b["instructions"] = out
    return _json.dumps(bir).encode()


def _pe_touch(nc, producers):
    """Advance PE's vector clock past each producer, one sem at a time."""
    for p in producers:
        n = nc.tensor.nop()
        add_dep_helper(n.ins, p.ins, sync=True, reason="pe-wait-absorber")


F32 = mybir.dt.float32
BF16 = mybir.dt.bfloat16
AF = mybir.ActivationFunctionType

D = 2048      # d_model
TOKS = 2048   # tokens per batch
QD = 512      # q dims per core
DH = 64
NCK = 16      # d_model chunks of 128
TT = 512      # token tile for projections
NTT = TOKS // TT
QTILE = 512
NQT = TOKS // QTILE
NKC = TOKS // 128   # key chunks of 128
V65 = DH + 1        # V' stationary width incl. ones column
SCALE = DH ** -0.5  # 0.125

# local head order within a core: chunk j holds heads (j, j+4)
HEAD_ORDER = [0, 4, 1, 5, 2, 6, 3, 7]


def _build():
    nc = bass.Bass()
    # all big inputs arrive pre-arranged by the host as their exact SBUF
    # images ([128 partitions, free]) so each loads in ONE DMA dispatch —
    # per-dispatch descriptor generation (~600ns each) was gating the
    # projection phase, not bandwidth
    xT = nc.declare_dram_parameter("xT", [128, NCK * TOKS], BF16,
                                   isOutput=False)
    wqT = nc.declare_dram_parameter("wqT", [128, NCK * QD], BF16,
                                    isOutput=False)
    wkT = nc.declare_dram_parameter("wkT", [128, NCK * 128], BF16,
                                    isOutput=False)
    wvT = nc.declare_dram_parameter("wvT", [128, NCK * 128], BF16,
                                    isOutput=False)
    woT = nc.declare_dram_parameter("woT", [128, 4 * D], BF16,
                                    isOutput=False)
    bq4 = nc.declare_dram_parameter("bq4", [128, 4], F32, isOutput=False)
    bkT = nc.declare_dram_parameter("bkT", [128, 1], F32, isOutput=False)
    bvT = nc.declare_dram_parameter("bvT", [128, 1], F32, isOutput=False)
    eye = nc.declare_dram_parameter("eye", [128, 128], BF16, isOutput=False)
    out = nc.declare_dram_parameter("out", [TOKS, D], BF16, isOutput=True)
    scr = nc.declare_dram_parameter("scr", [4, 2 * QTILE], BF16,
                                    isOutput=True)

    with TileContext(nc) as tc, \
            nc.allow_low_precision(reason="bf16 kernel; tol 2e-2"), \
            tc.tile_pool(name="persist", bufs=1) as pp:
        if True:
            xT_sb = pp.tile([128, NCK * TOKS], BF16, tag="xT")
            wq_sb = pp.tile([128, NCK * QD], BF16, tag="wq")
            wk_sb = pp.tile([128, NCK * 128], BF16, tag="wk")
            wv_sb = pp.tile([128, NCK * 128], BF16, tag="wv")
            wo_sb = pp.tile([128, 4 * D], BF16, tag="wo")
            qT = pp.tile([128, 4 * TOKS], BF16, tag="qT")
            kT = pp.tile([128, TOKS], BF16, tag="kT")
            vT_sb = pp.tile([128, TOKS], BF16, tag="vT")
            vp0 = pp.tile([128, NKC * V65], BF16, tag="vp0")
            vp1 = pp.tile([128, NKC * V65], BF16, tag="vp1")
            ones65 = pp.tile([65, DH], BF16, tag="ones65")
            eye_sb = pp.tile([128, 128], BF16, tag="eye")
            bq_sb = pp.tile([128, 4], F32, tag="bq")
            bk_sb = pp.tile([128, 1], F32, tag="bk")
            bv_sb = pp.tile([128, 1], F32, tag="bv")

            # input loads: wk/wv first (small, gate the first K/V matmuls),
            # then xT in 16 single-chunk slices round-robin over FOUR DMA
            # queues so chunks land well ahead of the projection loop's
            # consumption order; wq/wo (big, needed later) go last.
            # queue plan (3 DMA queues, each ~130GB/s, depth ~4): xT chunks
            # round-robin in consumption order; wk/wv ahead of their first
            # use; wq is jg-major so Q(tt0,j0) only needs the first quarter
            # (lands right as K/V projections finish); biases/eye/wo slot
            # into remaining queue time before their ~30-100us uses.
            # xT arrives tt-major ([128, tt(4) x ck(16) x 512]) in 16
            # quarter-slab dispatches so the K/V projections (also
            # tt-major now) stream behind the DMA with no bulk wait
            # gpsimd/scalar carry only xT slabs (slab0/slab1 land first);
            # sync carries wk+wv up front, then every third slab, then the
            # later-needed weights.  wq's j0 block is slotted so it lands
            # just before Q(tt0,j0) follows the K/V projections.
            nc.sync.dma_start(out=wk_sb[:, :], in_=wkT[:, :])
            SLAB = 4 * TT  # 4 ck-slices of one tt
            xqueues = [nc.gpsimd, nc.scalar, nc.sync]
            for s in range(16):
                if s == 0:
                    nc.gpsimd.dma_start(out=xT_sb[:, 0:SLAB],
                                        in_=xT[:, 0:SLAB])
                    nc.gpsimd.dma_start(out=wv_sb[:, :], in_=wvT[:, :])
                    continue
                xqueues[s % 3].dma_start(
                    out=xT_sb[:, s * SLAB:(s + 1) * SLAB],
                    in_=xT[:, s * SLAB:(s + 1) * SLAB])
                if s == 5:
                    nc.scalar.dma_start(out=wq_sb[:, 0:NCK * 128],
                                        in_=wqT[:, 0:NCK * 128])
                    nc.sync.dma_start(out=bk_sb[:, :], in_=bkT[:, :])
                    nc.gpsimd.dma_start(out=bv_sb[:, :], in_=bvT[:, :])
                    nc.sync.dma_start(out=eye_sb[:, :], in_=eye[:, :])
            nc.scalar.dma_start(out=bq_sb[:, :], in_=bq4[:, :])
            nc.gpsimd.dma_start(out=wq_sb[:, NCK * 128:],
                                in_=wqT[:, NCK * 128:])
            nc.sync.dma_start(out=wo_sb[:, :], in_=woT[:, :])
            # preload the Exp activation table so the first real exp
            # doesn't eat the ~1.3us table-load latency mid-pipeline
            warm = pp.tile([1, 1], F32, tag="warm")
            nc.vector.memset(warm[:, :], 0.0)
            nc.scalar.activation(warm[:, :], warm[:, :], AF.Exp)
            # ones columns of vp0/vp1 and ones65: memset whole tiles to 1.0;
            # V-proj writes only the 64 data columns of each 65-chunk.
            nc.vector.memset(vp0[:, :], 1.0)
            nc.vector.memset(vp1[:, :], 1.0)
            nc.vector.memset(ones65[:, :], 1.0)

            # ---------------- projection phase ----------------
            # K and vT interleaved per contraction chunk so the PE has
            # work as soon as each xT chunk's DMA lands (8 PSUM banks:
            # one accumulator per token tile per projection).  Q is done
            # here only for (tt=0, j=0); the rest streams into the
            # attention rounds via the quota queue.
            with tc.tile_pool(name="proj", bufs=1) as jp, \
                 tc.tile_pool(name="projps", bufs=1, space="PSUM") as jpp:
                # tt-major: each 512-token slab's K/V accumulation streams
                # right behind its DMA slices, then its V' transposes run,
                # so the PE starts ~10us in and never bulk-waits on xT
                kps = [jpp.tile([128, TT], F32, tag="kp", bufs=4,
                                name="kp%d" % t) for t in range(NTT)]
                vts = [jpp.tile([128, TT], F32, tag="vt", bufs=4,
                                name="vt%d" % t) for t in range(NTT)]
                TTW = NCK * TT  # cols per tt block in the tt-major image
                for tt in range(NTT):
                    for ck in range(NCK):
                        nc.tensor.matmul(
                            kps[tt][:, :],
                            wk_sb[:, ck * 128:(ck + 1) * 128],
                            xT_sb[:, tt * TTW + ck * TT:
                                  tt * TTW + (ck + 1) * TT],
                            start=(ck == 0), stop=(ck == NCK - 1))
                        nc.tensor.matmul(
                            vts[tt][:, :],
                            wv_sb[:, ck * 128:(ck + 1) * 128],
                            xT_sb[:, tt * TTW + ck * TT:
                                  tt * TTW + (ck + 1) * TT],
                            start=(ck == 0), stop=(ck == NCK - 1))
                    nc.vector.tensor_scalar_add(
                        kT[:, tt * TT:(tt + 1) * TT], kps[tt][:, :],
                        bk_sb[:, 0:1])
                    nc.vector.tensor_scalar_add(
                        vT_sb[:, tt * TT:(tt + 1) * TT], vts[tt][:, :],
                        bv_sb[:, 0:1])
                    # V' natural layout [keys, v-dims] via PE transpose
                    for kc in range(4 * tt, 4 * tt + 4):
                        tp = jpp.tile([128, 128], BF16, tag="vt", bufs=4)
                        nc.tensor.transpose(
                            tp[:, :], vT_sb[:, kc * 128:(kc + 1) * 128],
                            eye_sb[:, :])
                        nc.vector.tensor_copy(
                            vp0[:, kc * V65: kc * V65 + DH], tp[:, 0:DH])
                        nc.vector.tensor_copy(
                            vp1[:, kc * V65: kc * V65 + DH],
                            tp[:, DH:128])
                # Q(tt0,j0) reuses the kp banks (tag sharing — no
                # pool-close barrier before attention)
                ps = jpp.tile([128, TT], F32, tag="kp", bufs=4)
                for ck in range(NCK):
                    nc.tensor.matmul(
                        ps[:, :],
                        wq_sb[:, ck * 128: (ck + 1) * 128],
                        xT_sb[:, ck * TT: (ck + 1) * TT],
                        start=(ck == 0), stop=(ck == NCK - 1))
                nc.vector.tensor_scalar_add(
                    qT[:, 0:TT], ps[:, :], bq_sb[:, 0:1])

            # ---------------- attention + out-projection ----------------
            with tc.tile_pool(name="attn", bufs=1) as ap, \
                 tc.tile_pool(name="attnps", bufs=1, space="PSUM") as app:

                def op_store(qt, n, m, op, engs=None, osbufs=4):
                    osb = ap.tile([128, 512], BF16, tag="osb", bufs=osbufs)
                    nc.vector.tensor_copy(osb[:, :], op[:, :])
                    # alternate store queues so writeback never queues
                    # behind the normalize chain's small sync-queue DMAs
                    engs = engs or (nc.sync, nc.gpsimd)
                    eng = engs[(n * 4 + m) % len(engs)]
                    eng.dma_start(
                        out=out[qt * QTILE + m * 128:
                                qt * QTILE + (m + 1) * 128,
                                n * 512:(n + 1) * 512],
                        in_=osb[:, :])

                def op_group(qt, n, m, oT_t, pool, tag, bufs):
                    # out[qt*512+m*128 : +128, n*512 : +512] partial
                    op = pool.tile([128, TT], F32, tag=tag, bufs=bufs)
                    for j in range(4):
                        nc.tensor.matmul(
                            op[:, :],
                            oT_t[:, j * QTILE + m * 128:
                                 j * QTILE + (m + 1) * 128],
                            wo_sb[:, j * D + n * 512: j * D + (n + 1) * 512],
                            start=(j == 0), stop=(j == 3))
                    op_store(qt, n, m, op)

                def spair(qt, j, kc):
                    Sp = app.tile([128, 2 * QTILE], F32, tag="S", bufs=2)
                    qs = j * TOKS + qt * QTILE
                    nc.tensor.matmul(
                        Sp[:, 0:QTILE],
                        kT[0:64, kc * 128:(kc + 1) * 128],
                        qT[0:64, qs:qs + QTILE],
                        start=True, stop=True)
                    nc.tensor.matmul(
                        Sp[:, QTILE:2 * QTILE],
                        kT[64:128, kc * 128:(kc + 1) * 128],
                        qT[64:128, qs:qs + QTILE],
                        start=True, stop=True)
                    return Sp

                # Normalize oT = num/den.  Split into deferred stages so
                # the slow parts never block the PE queue or the opj banks:
                #   kc==15: stage PSUM out to SBUF (bf16, incl. den row),
                #           spread den [1,1024] -> [8,128] via DMA so the
                #           multi-pass DVE reciprocal costs ~0.9us not 6.5
                #   +1:     reciprocal on the spread layout
                #   +2:     cast + gather back to [1,1024] + half-1
                #           partition-shift DMA
                #   +7:     PE den broadcast, bcs copy, muls into oT
                def norm_stage1(opj0, opj1, pool=None):
                    pool = pool or ap
                    stg65 = pool.tile([65, 2 * QTILE], BF16, tag="stg65",
                                      bufs=2)
                    nc.vector.tensor_copy(stg65[:, 0:QTILE], opj0[0:65, :])
                    nc.vector.tensor_copy(stg65[:, QTILE:2 * QTILE],
                                          opj1[0:65, :])
                    dsp = pool.tile([8, 128], BF16, tag="dsp", bufs=2)
                    nc.sync.dma_start(out=dsp[0:8, 0:128],
                                      in_=stg65[64:65, 0:2 * QTILE])
                    return stg65, dsp

                def norm_recip(ctx, pool=None):
                    pool = pool or ap
                    stg65, dsp = ctx["stg"]
                    rdf = pool.tile([8, 128], F32, tag="rdf", bufs=2)
                    nc.vector.reciprocal(rdf[:, :], dsp[:, :])
                    ctx["rdf"] = rdf

                def norm_gather(ctx, pool=None):
                    pool = pool or ap
                    stg65, dsp = ctx["stg"]
                    rdb = pool.tile([8, 128], BF16, tag="rdb", bufs=2)
                    nc.vector.tensor_copy(rdb[:, :], ctx["rdf"][:, :])
                    # reciprocal row -> DRAM scratch -> broadcast-read to all
                    # 128 partitions: replaces the two PE broadcast matmuls
                    s = norm_gather.seg % 4
                    norm_gather.seg += 1
                    nc.sync.dma_start(out=scr[s:s + 1, :], in_=rdb[0:8, 0:128])
                    bcs = pool.tile([128, 2 * QTILE], BF16, tag="bcsb",
                                    bufs=2)
                    nc.sync.dma_start(
                        out=bcs[:, :],
                        in_=scr[s:s + 1, :].partition_broadcast(128))
                    stgB = pool.tile([128, QTILE], BF16, tag="stgB", bufs=2)
                    nc.sync.dma_start(out=stgB[64:128, :],
                                      in_=stg65[0:64, QTILE:2 * QTILE])
                    ctx["bcs"], ctx["stgB"] = bcs, stgB
                norm_gather.seg = 0

                def norm_stage2(ctx, oT_t, j, pspool=None, pstag="op",
                                psbufs=1, sbpool=None):
                    stg65, _ = ctx["stg"]
                    bcs, stgB = ctx["bcs"], ctx["stgB"]
                    nc.vector.tensor_mul(
                        oT_t[0:64, j * QTILE:(j + 1) * QTILE],
                        stg65[0:64, 0:QTILE], bcs[0:64, 0:QTILE])
                    nc.vector.tensor_mul(
                        oT_t[64:128, j * QTILE:(j + 1) * QTILE],
                        stgB[64:128, :], bcs[64:128, QTILE:2 * QTILE])

                # software-pipelined emission: per round, ACT gets exp(k)
                # first, then PE gets S-pair(k+1), an out-proj filler
                # group, the AV-pair(k), and any deferred normalize stages
                # scheduled for this round.
                rounds = [(qt, j, kc) for qt in range(NQT)
                          for j in range(4) for kc in range(NKC)]
                sched = {}  # round index -> [closure]
                pending, prev_ops, pi = [], [], 0
                oT_sb = None
                opj0 = opj1 = None
                # deferred Q-projection jobs (token tiles 1..3), paced into
                # attention rounds: 2/round in qt0 (no out-proj fillers
                # there), 1/round in qt1-2 skipping filler rounds, so
                # per-round PE work stays under the ACT exp cadence
                qjobs = [(0, jg, ck) for jg in range(1, 4)
                         for ck in range(NCK)]
                qjobs += [(tt, jg, ck) for tt in range(1, NTT)
                          for jg in range(4) for ck in range(NCK)]
                qjobs.reverse()  # pop() from the front order
                qp2 = None

                def emit_qmm(job):
                    nonlocal qp2
                    tt, jg, ck = job
                    if ck == 0:
                        qp2 = app.tile([128, TT], F32, tag="qp2", bufs=1)
                    nc.tensor.matmul(
                        qp2[:, :],
                        wq_sb[:, jg * (NCK * 128) + ck * 128:
                              jg * (NCK * 128) + (ck + 1) * 128],
                        xT_sb[:, tt * TTW + ck * TT:
                              tt * TTW + (ck + 1) * TT],
                        start=(ck == 0), stop=(ck == NCK - 1))
                    if ck == NCK - 1:
                        nc.vector.tensor_scalar_add(
                            qT[:, jg * TOKS + tt * TT:
                               jg * TOKS + (tt + 1) * TT],
                            qp2[:, :], bq_sb[:, jg:jg + 1])

                Sp_cur = spair(*rounds[0])
                for i, (qt, j, kc) in enumerate(rounds):
                    if j == 0 and kc == 0:
                        # persist pool: read by the tail-pool out-proj after
                        # the attn SBUF pool closes
                        oT_sb = pp.tile([128, 4 * QTILE], BF16, tag="oTsb",
                                        bufs=2)
                        prev_ops, pi = pending, 0
                        pending = [(qt, n, m, oT_sb)
                                   for n in range(4) for m in range(4)]
                    if kc == 0:
                        opj0 = app.tile([65, QTILE], F32, tag="opj0", bufs=1)
                        opj1 = app.tile([65, QTILE], F32, tag="opj1", bufs=1)
                    E = ap.tile([128, 2 * QTILE], BF16, tag="E", bufs=4)
                    nc.scalar.activation(
                        E[:, :], Sp_cur[:, :], AF.Exp, scale=SCALE)
                    Sp_nxt = (spair(*rounds[i + 1])
                              if i + 1 < len(rounds) else None)
                    r = j * NKC + kc
                    did_op = False
                    if r >= 8 and r % 3 == 2 and pi < len(prev_ops):
                        op_group(*prev_ops[pi], app, "op", 1)
                        pi += 1
                        did_op = True
                    for fn in sched.pop(i, ()):
                        fn()
                    # 2/round in qt0 (no fillers there), 1/round in
                    # non-filler rounds after — keeps per-round PE work
                    # near the ACT exp cadence while finishing each token
                    # tile's Q before its attention q-tile starts
                    quota = 2 if qt == 0 else (0 if did_op else 1)
                    while quota > 0 and qjobs:
                        emit_qmm(qjobs.pop())
                        quota -= 1
                    # AV: stationary [v-dims | ones] -> rows 0:65
                    # (row 64 = denominator)
                    nc.tensor.matmul(
                        opj0[0:V65, :],
                        vp0[:, kc * V65:(kc + 1) * V65],
                        E[:, 0:QTILE],
                        start=(kc == 0), stop=(kc == NKC - 1))
                    nc.tensor.matmul(
                        opj1[0:V65, :],
                        vp1[:, kc * V65:(kc + 1) * V65],
                        E[:, QTILE:2 * QTILE],
                        start=(kc == 0), stop=(kc == NKC - 1))
                    if kc == NKC - 1:
                        if i == len(rounds) - 1:
                            # last segment: stage1 tiles live in the persist
                            # pool so the remaining stages can run in the
                            # tail pool after the attn pools close
                            tail_ctx = {"stg": norm_stage1(opj0, opj1,
                                                           pool=pp)}
                            tail_j = j
                        else:
                            ctx = {"stg": norm_stage1(opj0, opj1)}
                            sched.setdefault(i + 1, []).append(
                                lambda c=ctx: norm_recip(c))
                            sched.setdefault(i + 2, []).append(
                                lambda c=ctx: norm_gather(c))
                            sched.setdefault(i + 7, []).append(
                                lambda c=ctx, t=oT_sb, jj=j:
                                norm_stage2(c, t, jj))
                    Sp_cur = Sp_nxt
                # flush deferred normalize stages of the last segment
                for idx in sorted(sched):
                    for fn in sched[idx]:
                        fn()
                while pi < len(prev_ops):
                    op_group(*prev_ops[pi], app, "op", 1)
                    pi += 1
            # tail: qt3's out-projection. The PE queue is in-order, so the
            # j3-normalize chain's DVE/DMA hops go first, then 7 groups'
            # j0..j2 partial matmuls keep the PE streaming while that chain
            # completes; stage2(j3) then lands with no PE stall, followed by
            # the j3 finishers and the remaining 9 full groups.
            with tc.tile_pool(name="tail", bufs=1) as ap, \
                 tc.tile_pool(name="tailps", bufs=1, space="PSUM") as tpp:
                norm_recip(tail_ctx, pool=ap)
                norm_gather(tail_ctx, pool=ap)

                def op_partial(args):
                    (qt, n, m, oT_t) = args
                    op = tpp.tile([128, TT], F32, tag="opt", bufs=8)
                    for j in range(3):
                        nc.tensor.matmul(
                            op[:, :],
                            oT_t[:, j * QTILE + m * 128:
                                 j * QTILE + (m + 1) * 128],
                            wo_sb[:, j * D + n * 512: j * D + (n + 1) * 512],
                            start=(j == 0), stop=False)
                    return op

                tailq = (nc.sync, nc.gpsimd, nc.scalar)
                ops1 = [op_partial(a) for a in pending[:8]]
                # bcj(j3) sits here in the in-order PE stream: by now the
                # 18 partial matmuls above have covered the recip chain
                norm_stage2(tail_ctx, oT_sb, tail_j, pspool=tpp,
                            pstag="bcj", psbufs=1, sbpool=ap)
                for (qt, n, m, oT_t), op in zip(pending[:8], ops1):
                    nc.tensor.matmul(
                        op[:, :],
                        oT_t[:, 3 * QTILE + m * 128:
                             3 * QTILE + (m + 1) * 128],
                        wo_sb[:, 3 * D + n * 512: 3 * D + (n + 1) * 512],
                        start=False, stop=True)
                    op_store(qt, n, m, op, engs=tailq, osbufs=6)
                for (qt, n, m, oT_t) in pending[8:]:
                    op = tpp.tile([128, TT], F32, tag="opt", bufs=8)
                    for j in range(4):
                        nc.tensor.matmul(
                            op[:, :],
                            oT_t[:, j * QTILE + m * 128:
                                 j * QTILE + (m + 1) * 128],
                            wo_sb[:, j * D + n * 512: j * D + (n + 1) * 512],
                            start=(j == 0), stop=(j == 3))
                    op_store(qt, n, m, op, engs=tailq, osbufs=6)
    return nc


def _prep_inputs(x, Wq, bq, Wk, bk, Wv, bv, Wo, bo):
    """Build the 8 per-core input maps."""
    f = np.float32
    bf = ml_dtypes.bfloat16
    x = np.asarray(x, f)
    Wq, bq = np.asarray(Wq, f), np.asarray(bq, f)
    Wk, bk = np.asarray(Wk, f), np.asarray(bk, f)
    Wv, bv = np.asarray(Wv, f), np.asarray(bv, f)
    Wo = np.asarray(Wo, f)
    # per-core head-dim permutation within the group's 512 q dims
    perm = np.concatenate([
        np.arange(h * DH, (h + 1) * DH) for h in HEAD_ORDER])
    eye = np.eye(128, dtype=f).astype(bf)
    in_maps = []
    for c in range(8):
        b, g = divmod(c, 4)
        wq_g = Wq[g * QD:(g + 1) * QD, :][perm, :]     # (512, 2048)
        bq_g = bq[g * QD:(g + 1) * QD][perm]
        wo_g = Wo[:, g * QD:(g + 1) * QD].T[perm, :]   # (512, 2048)
        def sbimg(a):
            # [NCK*128, w] -> SBUF image [128, NCK*w]: chunk ck of 128
            # DRAM rows becomes columns [ck*w, (ck+1)*w) on partition p
            a = np.asarray(a, f).astype(bf)
            n, w = a.shape
            return np.ascontiguousarray(
                a.reshape(n // 128, 128, w).transpose(1, 0, 2)
                .reshape(128, (n // 128) * w))

        # wq image is jg-major: [128, jg(4) x ck(16) x 128] so the j0 block
        # (first quarter) alone unblocks Q(tt0, j0)
        wq_jg = (np.asarray(wq_g.T, f).astype(bf)
                 .reshape(16, 128, 4, 128).transpose(1, 2, 0, 3)
                 .reshape(128, 4 * 16 * 128))
        # xT image is tt-major: [128, tt(4) x ck(16) x 512]
        xT_tt = (x[b].T.astype(bf).reshape(16, 128, 4, 512)
                 .transpose(1, 2, 0, 3).reshape(128, 4 * 16 * 512))
        in_maps.append({
            "xT": np.ascontiguousarray(xT_tt),
            "wqT": np.ascontiguousarray(wq_jg),
            "wkT": sbimg(Wk[g * 128:(g + 1) * 128, :].T),
            "wvT": sbimg(Wv[g * 128:(g + 1) * 128, :].T),
            "woT": sbimg(wo_g),
            "bq4": np.ascontiguousarray(bq_g.reshape(4, 128).T),
            "bkT": np.ascontiguousarray(bk[g * 128:(g + 1) * 128, None]),
            "bvT": np.ascontiguousarray(bv[g * 128:(g + 1) * 128, None]),
            "eye": eye,
        })
    return in_maps


def run(inputs, trace=False, **kw):
    nc = _build()
    _orig_tjb = nc.to_json_bytes
    nc.to_json_bytes = lambda: _split_matmul_waits(_orig_tjb())
    in_maps = _prep_inputs(**inputs)
    res = run_bass_kernel_spmd(nc, in_maps, list(range(8)), trace=trace, **kw)
    bo = np.asarray(inputs["bo"], np.float32)
    y = np.empty((2, TOKS, D), np.float32)
    for b in range(2):
        acc = res.results[4 * b]["out"].astype(np.float32)
        for g in range(1, 4):
            acc = acc + res.results[4 * b + g]["out"].astype(np.float32)
        y[b] = acc + bo[None, :]
    return y, res


def kernel(**inputs):
    y, _ = run(inputs, trace=False)
    return y

